# revision 8
# baseline (speedup 1.0000x reference)
"""Trainium2 Bass kernel for nn_AdjacencyMatrix (gnn_message_passing).

Computes G = softmax_w( (z @ Wt^T + bt) @ (z @ Wp^T + bp)^T ) per (n,t) graph,
data-parallel over the 128 (n,t) graphs across 8 NeuronCores (16 graphs/core).

Math notes:
  S = theta @ phi^T with theta = Z Wt^T + 1 bt^T, phi = Z Wp^T + 1 bp^T.
  Expanding, S = P Q^T + u 1^T + 1 r^T + const, where P = Z Wt^T, Q = Z Wp^T,
  u[v] row-constant terms drop under softmax over w, and r = Z (Wp^T bt).
  We fold r into the phi projection by augmenting Wp^T with the column
  q = Wp^T bt (device computes row 64 = Z q = r), and add a ones-row to the
  theta-side stationary so the K=65 S-matmul adds 1*r[w] directly.

Per-core device pipeline (per graph):
  DMA z [1024,256] -> PE-transpose to Z^T -> projections (K=c, f32r)
  -> S tiles [128v,1024w] (K=65, f32r) -> ScalarE exp(+row-sum accumulate)
  -> VectorE reciprocal + scale -> DMA out.
"""

import os
import sys

if "/opt/trn_rl_repo" not in sys.path:
    sys.path.insert(0, "/opt/trn_rl_repo")

import numpy as np

N_CORES = 8
NT = 128            # total (n,t) graphs
G = NT // N_CORES   # graphs per core
V = 1024
C = 256
O = 64
OA = O + 1          # augmented rows (bias trick)

LAST_RESULT = None
_NC_CACHE = {}


def _build_nc():
    import concourse.bacc as bacc
    import concourse.tile as tile
    from concourse import mybir
    from concourse.masks import make_identity

    f32 = mybir.dt.float32
    f32r = mybir.dt.float32r
    EXP = mybir.ActivationFunctionType.Exp
    IDENT = mybir.ActivationFunctionType.Identity

    nc = bacc.Bacc("TRN2", target_bir_lowering=False, debug=False,
                   num_devices=N_CORES)
    z_d = nc.dram_tensor("z", [G, V, C], f32, kind="ExternalInput")
    w_d = nc.dram_tensor("w", [128, 2, 2, OA], f32, kind="ExternalInput")
    out_d = nc.dram_tensor("out", [G, V, V], f32, kind="ExternalOutput")

    with tile.TileContext(nc) as tc:
        with (
            tc.tile_pool(name="consts", bufs=1) as consts,
            tc.tile_pool(name="zn", bufs=2) as p_zn,
            tc.tile_pool(name="zt", bufs=2) as p_zt,
            tc.tile_pool(name="th", bufs=2) as p_th,
            tc.tile_pool(name="ph", bufs=2) as p_ph,
            tc.tile_pool(name="ex", bufs=3) as p_ex,
            tc.tile_pool(name="ot", bufs=3) as p_ot,
            tc.tile_pool(name="sm", bufs=6) as p_sm,
            tc.tile_pool(name="pt", bufs=2, space="PSUM") as p_pt,
            tc.tile_pool(name="pp", bufs=2, space="PSUM") as p_pp,
            tc.tile_pool(name="ps", bufs=2, space="PSUM") as p_ps,
        ):
            ident = consts.tile([128, 128], f32)
            make_identity(nc, ident[:])
            w_f32 = consts.tile([128, 2, 2, OA], f32)
            nc.sync.dma_start(out=w_f32, in_=w_d.ap())
            w_sb = consts.tile([128, 2, 2, OA], f32r)
            nc.vector.tensor_copy(out=w_sb, in_=w_f32)
            # bias vector for theta eviction: +1.0 on row 64 (the ones-row)
            bias_th = consts.tile([OA, 1], f32)
            nc.vector.memset(bias_th[0:O], 0.0)
            nc.vector.memset(bias_th[O:OA], 1.0)

            z_ap = z_d.ap()
            o_ap = out_d.ap()

            for g in range(G):
                zn = p_zn.tile([128, 8, C], f32)
                nc.sync.dma_start(
                    out=zn, in_=z_ap[g].rearrange("(vo p) c -> p vo c", p=128)
                )

                # Z^T via PE transposes: zt[:, kc, v] = z[v, kc*128 + p]
                zt = p_zt.tile([128, 2, V], f32r)
                for kc in range(2):
                    for vh in range(2):
                        pt = p_pt.tile([128, 4, 128], f32)
                        for q in range(4):
                            vo = vh * 4 + q
                            nc.tensor.transpose(
                                pt[:, q, :],
                                zn[:, vo, kc * 128:(kc + 1) * 128],
                                ident,
                            )
                        nc.vector.tensor_copy(
                            out=zt[:, kc, vh * 512:(vh + 1) * 512].rearrange(
                                "p (a b) -> p a b", a=4
                            ),
                            in_=pt,
                        )

                # Projections: th/ph[o, v] (o on partitions), K = c
                th = p_th.tile([OA, V], f32r)
                ph = p_ph.tile([OA, V], f32r)
                for j, dst in ((0, th), (1, ph)):
                    for vc in range(2):
                        pp = p_pp.tile([OA, 512], f32)
                        for kc in range(2):
                            nc.tensor.matmul(
                                pp,
                                lhsT=w_sb[:, j, kc, :],
                                rhs=zt[:, kc, vc * 512:(vc + 1) * 512],
                                start=(kc == 0),
                                stop=(kc == 1),
                            )
                        if j == 0:
                            # evict + bias: row 64 = 0 (zero weight col) + 1.0
                            nc.scalar.activation(
                                out=dst[:, vc * 512:(vc + 1) * 512],
                                in_=pp,
                                func=IDENT,
                                bias=bias_th[:],
                                scale=1.0,
                            )
                        else:
                            nc.vector.tensor_copy(
                                out=dst[:, vc * 512:(vc + 1) * 512], in_=pp
                            )

                # S = th^T @ ph (K=65) then row softmax
                ot = None
                for vo in range(8):
                    ps = p_ps.tile([128, V], f32)
                    for wc in range(2):
                        nc.tensor.matmul(
                            ps[:, wc * 512:(wc + 1) * 512],
                            lhsT=th[:, vo * 128:(vo + 1) * 128],
                            rhs=ph[:, wc * 512:(wc + 1) * 512],
                            start=True,
                            stop=True,
                        )
                    ex = p_ex.tile([128, V], f32)
                    sm = p_sm.tile([128, 2], f32)
                    nc.scalar.activation(
                        out=ex, in_=ps, func=EXP, accum_out=sm[:, 0:1]
                    )
                    nc.vector.reciprocal(out=sm[:, 1:2], in_=sm[:, 0:1])
                    if vo % 2 == 0:
                        ot = p_ot.tile([128, 2, V], f32)
                    nc.vector.tensor_scalar_mul(ot[:, vo % 2, :], ex, sm[:, 1:2])
                    if vo % 2 == 1:
                        nc.sync.dma_start(
                            out=o_ap[g].rearrange("(vp p) x -> p vp x", p=128)[
                                :, vo - 1:vo + 1, :
                            ],
                            in_=ot,
                        )

    nc.compile()
    return nc


def _get_nc():
    if "nc" not in _NC_CACHE:
        _NC_CACHE["nc"] = _build_nc()
    return _NC_CACHE["nc"]


def kernel(z, theta_w, theta_b, phi_w, phi_b):
    from concourse.bass_utils import run_bass_kernel_spmd

    global LAST_RESULT
    z = np.asarray(z, dtype=np.float32)
    theta_w = np.asarray(theta_w, dtype=np.float32)
    theta_b = np.asarray(theta_b, dtype=np.float32)
    phi_w = np.asarray(phi_w, dtype=np.float32)
    phi_b = np.asarray(phi_b, dtype=np.float32)

    n, t = z.shape[0], z.shape[1]
    zf = z.reshape(NT, V, C)

    # Augmented transposed weights: wt[j, c, o]; j=0 theta (col 64 unused,
    # overwritten by device ones-row), j=1 phi (col 64 = q = Wp^T bt).
    wt = np.zeros((2, C, OA), dtype=np.float32)
    wt[0, :, :O] = theta_w.T
    wt[1, :, :O] = phi_w.T
    wt[1, :, O] = phi_w.T @ theta_b
    # SBUF layout [p, j, kc, o] with c = kc*128 + p
    w_host = np.ascontiguousarray(
        wt.reshape(2, 2, 128, OA).transpose(2, 0, 1, 3)
    )

    nc = _get_nc()
    in_maps = [
        {"z": np.ascontiguousarray(zf[i * G:(i + 1) * G]), "w": w_host}
        for i in range(N_CORES)
    ]
    res = run_bass_kernel_spmd(nc, in_maps, core_ids=list(range(N_CORES)))
    LAST_RESULT = res
    out = np.concatenate(
        [res.results[i]["out"] for i in range(N_CORES)], axis=0
    )
    return out.reshape(n, t, V, V)
